# revision 7
# baseline (speedup 1.0000x reference)
"""Trainium2 Bass kernel for nn_Decoder_36953898615460.

recon[B, D] = einsum('lbf,lfd->bd', acts[:n], W[:n]) + sum(bias[:n], 0)

Strategy (row-parallel over F, 8 NeuronCores; TimelineSim 400.6 us vs the
563.3 us HW baseline):
  - Shard the contraction dim F across 8 cores: core r owns F columns
    [r*768, (r+1)*768)  ->  local contraction K_loc = n*768 (9216 for n=12).
  - Host prep: inputs cast to bf16 (rel err 3.8e-3 << 2e-2 tol); acts shard
    transposed to [K_loc, B]; W shard reshaped to [K_loc, D]; bias -> [D, n].
  - bf16 halves HBM traffic vs fp32: per-core DMA-in ~52 MB (~145 us) vs the
    PE floor 29 GF / 78.6 TF/s = 369 us -> cleanly PE-bound; every real
    matmul issues at the 213 ns streaming floor in the cost model.
  - W (14.2 MB bf16) stays SBUF-resident: streamed once during block 0,
    interleaved per chunk with acts on the SP HWDGE queue (FIFO pacing; an
    ungated separate W stream floods the serialized DMA engines and starves
    acts). Chunk 0 goes in two sub-chunk DMAs (first matmul waits on half a
    chunk; finer splits lose to the ~1.2 us per-DMA dispatch overhead), with
    chunk-0 W on the ACT queue so both queues ramp in parallel.
  - PE warm-up: ~55 N=128 dummy matmuls on zeroed scratch bridge the initial
    DMA wait so the clock ramp (half rate for the first ~3 us of PE activity;
    HAM on real hw) burns before the real stream starts. N=128 keeps the
    per-MM issue floor (~60 cycles on silicon, uncharged by the cost model)
    below streaming time so the chain length matches on hw.
  - B processed in 4 blocks of 512 (one PSUM bank per d-subtile). Per block,
    the full K accumulation for each of the 6 subtiles lives in a single
    PSUM bank (72 accumulating matmuls, start/stop flags) -- no SBUF
    accumulator, no per-chunk vector adds.
  - Per-block bf16-wire ReduceScatter(add) overlaps the next block's
    compute; only the last block's RS (~17.5 us, launch-constant dominated)
    is exposed at the tail. Each block's last chunk runs m-outer with the
    evacuation issued right after that subtile's stop-matmul, so the RS
    starts ~3 us after the last matmul. The final subtile evacuates on DVE
    so the ACT queue pre-stages its partial write.
  - bias: each core adds sum_l(bias)/8 during PSUM->SBUF evacuation so the
    8-way reduce sums to +bias.
  - Output y is block-major bf16 [NBLK, 96, 512] (collectives cannot write
    IO tensors, so an internal reduced buffer is copied out; bf16 lets the
    last copy ride the idle SP queue -- dtype-converting DMA is Pool-only).
    Host reassembles the 8 shards and casts to fp32.
"""

import numpy as np
import ml_dtypes

import concourse.mybir as mybir
import concourse.tile as tile
from concourse import bacc
from concourse.bass import ts
from concourse.bass_utils import run_bass_kernel_spmd

NCORES = 8
B, F, D = 2048, 6144, 768
F_LOC = F // NCORES  # 768
P = 128
MD = D // P          # 6 d-subtiles
DR = D // NCORES     # 96 rows per rank after ReduceScatter
BN = 512             # B block width (= matmul moving free dim, one PSUM bank)
NBLK = B // BN       # 4
CK = 8               # preferred k-tiles (of 128) per DMA chunk
WARM_MMS = 55        # dummy matmuls bridging the initial DMA wait
CK0_SPLITS_A = (4,)  # chunk-0 acts sub-chunk boundaries (k-tiles)
CK0_SPLITS_W = (4,)  # chunk-0 W sub-chunk boundaries (k-tiles)
W_ACT = 0            # W chunks [1..W_ACT] ride the ACT queue

_nc_cache = {}
last_result = None  # BassKernelResults of the most recent run (for test harness)


def _build(n_layers: int):
    K_LOC = n_layers * F_LOC          # 9216 for n=12
    KT = K_LOC // P                   # 72 k-tiles
    ck = max(c for c in (CK, 6, 4, 3, 2, 1) if KT % c == 0)
    NCH = KT // ck                    # 9 chunks for n=12

    nc = bacc.Bacc(None, num_devices=NCORES)
    a_ext = nc.dram_tensor("a_t", [K_LOC, B], mybir.dt.bfloat16, kind="ExternalInput")
    w_ext = nc.dram_tensor("w", [K_LOC, D], mybir.dt.bfloat16, kind="ExternalInput")
    b_ext = nc.dram_tensor("bias_t", [D, n_layers], mybir.dt.float32, kind="ExternalInput")
    # block-major bf16 output (host casts to fp32): lets the final copy ride
    # the idle SP HWDGE queue (no dtype conversion, which only Pool DMA does)
    y_ext = nc.dram_tensor("y", [NBLK, DR, BN], mybir.dt.bfloat16, kind="ExternalOutput")

    # bf16 wire format for the reduce: halves RS payload + partial DMAs.
    # Adds ~1e-3 quantization on partials (total rel err stays ~2.6e-3).
    partials = [nc.dram_tensor(f"partial{b}", [D, BN], mybir.dt.bfloat16) for b in range(NBLK)]
    reduceds = [nc.dram_tensor(f"reduced{b}", [DR, BN], mybir.dt.bfloat16) for b in range(NBLK)]

    a_v = a_ext[:, :].rearrange("(ko p) b -> p ko b", p=P)  # [128, KT, B]
    w_v = w_ext[:, :].rearrange("(ko p) d -> p ko d", p=P)  # [128, KT, D]
    b_v = b_ext[:, :].rearrange("(mo p) l -> p mo l", p=P)  # [128, MD, n]

    with tile.TileContext(nc) as tc:
        with (
            tc.tile_pool(name="apool", bufs=3) as apool,
            tc.tile_pool(name="wpool", bufs=NCH) as wpool,
            tc.tile_pool(name="cpool", bufs=1) as cpool,
            tc.tile_pool(name="opool", bufs=3) as opool,
            tc.tile_pool(name="pspool", bufs=8, space="PSUM") as pspool,
        ):
            # bias8[p, mo] = sum_l bias[l, mo*128+p] / NCORES  (SWDGE: keep the
            # SP queue free for the first acts chunk)
            bias_t = cpool.tile([P, MD, n_layers], mybir.dt.float32)
            nc.gpsimd.dma_start(bias_t[:], b_v)
            bias8 = cpool.tile([P, MD], mybir.dt.float32)
            nc.vector.reduce_sum(bias8[:], bias_t[:], axis=mybir.AxisListType.X)
            nc.vector.tensor_scalar_mul(bias8[:], bias8[:], 1.0 / NCORES)

            # PE warm-up: a dense dummy-matmul chain that spans the first
            # acts/W DMA wait and ends only once real data is ready, so the
            # clock ramp (half-rate for the first ~3us of PE activity; HAM on
            # real hw) is burned before the real stream starts. The chain must
            # reach the first real matmul with no PE-idle gap or the ramp
            # resets.
            scratch = cpool.tile([P, P], mybir.dt.bfloat16)
            nc.vector.memset(scratch[:], 0)
            # N=128 dummies: wide enough that the per-MM issue floor (~60
            # cycles on real hw, uncharged by the cost model) stays below the
            # streaming time, so the chain length matches on silicon too
            ps_warm = pspool.tile([P, P], mybir.dt.float32, tag="ps", name="ps_warm")
            for i in range(WARM_MMS):
                nc.tensor.matmul(
                    ps_warm[:], scratch[:], scratch[:],
                    start=(i == 0), stop=(i == WARM_MMS - 1),
                )


            def evac(blk, m, ps_m):
                """PSUM -> SBUF (+bias/8) -> partial DRAM, alternating engines."""
                ob = opool.tile([P, BN], mybir.dt.bfloat16, tag="o", name=f"ob{blk}_{m}")
                if m in (0, 2, 5):  # m5 on DVE: the ACT queue then pre-stages
                    nc.vector.tensor_scalar_add(ob[:], ps_m[:], bias8[:, m : m + 1])
                else:               # its partial-DMA dispatch during the evac
                    nc.scalar.add(ob[:], ps_m[:], bias8[:, m : m + 1])
                nc.scalar.dma_start(partials[blk][ts(m, P), :], ob[:])

            w_tiles = []
            for blk in range(NBLK):
                b0 = blk * BN
                ps = [pspool.tile([P, BN], mybir.dt.float32, tag="ps", name=f"ps{blk}_{m}") for m in range(MD)]
                for c in range(NCH):
                    a_c = apool.tile([P, ck, BN], mybir.dt.bfloat16, tag="a")
                    if blk == 0:
                        w_c = wpool.tile([P, ck, D], mybir.dt.bfloat16, tag="w")
                        w_tiles.append(w_c)
                        if c == 0:
                            # chunk 0 in two sub-chunk DMAs: the first matmul
                            # waits on half the chunk, but per-DMA dispatch
                            # overhead (~1.2us) stays amortized so the stream
                            # feeds full-rate consumption. Chunk-0 W rides the
                            # ACT queue so both HWDGE queues ramp in parallel.
                            prev = 0
                            for h1 in [x for x in CK0_SPLITS_A if x < ck] + [ck]:
                                nc.sync.dma_start(a_c[:, prev:h1], a_v[:, prev:h1, b0 : b0 + BN])
                                prev = h1
                            prev = 0
                            for h1 in [x for x in CK0_SPLITS_W if x < ck] + [ck]:
                                nc.scalar.dma_start(w_c[:, prev:h1], w_v[:, prev:h1, :])
                                prev = h1
                        else:
                            # W for the first W_ACT chunks rides the ACT queue
                            # (fills DMA-device idle between SP dispatches);
                            # later chunks share the SP queue so FIFO order
                            # keeps the streams interleaved (an unbounded ACT
                            # stream floods the device and starves acts)
                            nc.sync.dma_start(a_c[:], a_v[:, c * ck : (c + 1) * ck, b0 : b0 + BN])
                            if c <= W_ACT:
                                nc.scalar.dma_start(w_c[:], w_v[:, c * ck : (c + 1) * ck, :])
                            else:
                                nc.sync.dma_start(w_c[:], w_v[:, c * ck : (c + 1) * ck, :])
                    else:
                        nc.sync.dma_start(a_c[:], a_v[:, c * ck : (c + 1) * ck, b0 : b0 + BN])
                        w_c = w_tiles[c]
                    if c < NCH - 1:
                        for k in range(ck):
                            for m in range(MD):
                                nc.tensor.matmul(
                                    ps[m][:],
                                    w_c[:, k, ts(m, P)],
                                    a_c[:, k],
                                    start=(c == 0 and k == 0),
                                    stop=False,
                                )
                    else:
                        # last chunk m-outer: each subtile's accumulation
                        # finishes early, its evacuation overlaps the rest
                        for m in range(MD):
                            for k in range(ck):
                                nc.tensor.matmul(
                                    ps[m][:],
                                    w_c[:, k, ts(m, P)],
                                    a_c[:, k],
                                    start=(c == 0 and k == 0),
                                    stop=(k == ck - 1),
                                )
                            evac(blk, m, ps[m])

                nc.gpsimd.collective_compute(
                    "ReduceScatter",
                    mybir.AluOpType.add,
                    replica_groups=[list(range(NCORES))],
                    ins=[partials[blk][:, :].opt()],
                    outs=[reduceds[blk][:, :].opt()],
                )
                if blk < NBLK - 1:
                    nc.gpsimd.dma_start(y_ext[blk], reduceds[blk][:, :])
                else:
                    # idle SP queue: dispatch pre-staged, waits only on RS sem
                    nc.sync.dma_start(y_ext[blk], reduceds[blk][:, :])
    nc.compile()
    return nc


def _get_nc(n_layers: int):
    if n_layers not in _nc_cache:
        _nc_cache[n_layers] = _build(n_layers)
    return _nc_cache[n_layers]


def kernel(acts: np.ndarray, W: np.ndarray, bias: np.ndarray, layer_idx) -> np.ndarray:
    global last_result
    n = int(layer_idx) + 1
    bf16 = ml_dtypes.bfloat16
    acts16 = np.asarray(acts, dtype=np.float32)[:n].astype(bf16)  # [n, B, F]
    W16 = np.asarray(W, dtype=np.float32)[:n].astype(bf16)        # [n, F, D]
    bias = np.asarray(bias, dtype=np.float32)[:n]                 # [n, D]

    nc = _get_nc(n)

    bias_t = np.ascontiguousarray(bias.T)  # [D, n], same on every core
    in_maps = []
    for r in range(NCORES):
        f0 = r * F_LOC
        # [n, B, F_LOC] -> [n, F_LOC, B] -> [K_loc, B]
        a_t = np.ascontiguousarray(
            acts16[:, :, f0 : f0 + F_LOC].transpose(0, 2, 1)
        ).reshape(n * F_LOC, B)
        w_r = np.ascontiguousarray(W16[:, f0 : f0 + F_LOC, :]).reshape(n * F_LOC, D)
        in_maps.append({"a_t": a_t, "w": w_r, "bias_t": bias_t})

    last_result = run_bass_kernel_spmd(nc, in_maps, core_ids=list(range(NCORES)))
    out = np.empty((D, B), dtype=np.float32)
    for r in range(NCORES):
        y_r = np.asarray(last_result.results[r]["y"]).astype(np.float32)  # [NBLK, DR, BN]
        for blk in range(NBLK):
            out[r * DR : (r + 1) * DR, blk * BN : (blk + 1) * BN] = y_r[blk]
    return np.ascontiguousarray(out.T)  # [B, D] float32


# revision 8
# speedup vs baseline: 1.0011x; 1.0011x over previous
"""Trainium2 Bass kernel for nn_Decoder_36953898615460.

recon[B, D] = einsum('lbf,lfd->bd', acts[:n], W[:n]) + sum(bias[:n], 0)

Strategy (row-parallel over F, 8 NeuronCores; TimelineSim 400.6 us vs the
563.3 us HW baseline):
  - Shard the contraction dim F across 8 cores: core r owns F columns
    [r*768, (r+1)*768)  ->  local contraction K_loc = n*768 (9216 for n=12).
  - Host prep: inputs cast to bf16 (rel err 3.8e-3 << 2e-2 tol); acts shard
    transposed to [K_loc, B]; W shard reshaped to [K_loc, D]; bias -> [D, n].
  - bf16 halves HBM traffic vs fp32: per-core DMA-in ~52 MB (~145 us) vs the
    PE floor 29 GF / 78.6 TF/s = 369 us -> cleanly PE-bound; every real
    matmul issues at the 213 ns streaming floor in the cost model.
  - W (14.2 MB bf16) stays SBUF-resident: streamed once during block 0,
    interleaved per chunk with acts on the SP HWDGE queue (FIFO pacing; an
    ungated separate W stream floods the serialized DMA engines and starves
    acts). Chunk 0 goes in two sub-chunk DMAs (first matmul waits on half a
    chunk; finer splits lose to the ~1.2 us per-DMA dispatch overhead), with
    chunk-0 W on the ACT queue so both queues ramp in parallel.
  - PE warm-up: ~55 N=128 dummy matmuls on zeroed scratch bridge the initial
    DMA wait so the clock ramp (half rate for the first ~3 us of PE activity;
    HAM on real hw) burns before the real stream starts. N=128 keeps the
    per-MM issue floor (~60 cycles on silicon, uncharged by the cost model)
    below streaming time so the chain length matches on hw.
  - B processed in 4 blocks of 512 (one PSUM bank per d-subtile). Per block,
    the full K accumulation for each of the 6 subtiles lives in a single
    PSUM bank (72 accumulating matmuls, start/stop flags) -- no SBUF
    accumulator, no per-chunk vector adds.
  - Per-block bf16-wire ReduceScatter(add) overlaps the next block's
    compute; only the last block's RS (~17.5 us, launch-constant dominated)
    is exposed at the tail. Each block's last chunk runs m-outer with the
    evacuation issued right after that subtile's stop-matmul, so the RS
    starts ~3 us after the last matmul. The final subtile evacuates on DVE
    so the ACT queue pre-stages its partial write.
  - bias: each core adds sum_l(bias)/8 during PSUM->SBUF evacuation so the
    8-way reduce sums to +bias.
  - Output y is block-major bf16 [NBLK, 96, 512] (collectives cannot write
    IO tensors, so an internal reduced buffer is copied out; bf16 lets the
    last copy ride the idle SP queue -- dtype-converting DMA is Pool-only).
    Host reassembles the 8 shards and casts to fp32.
"""

import numpy as np
import ml_dtypes

import concourse.mybir as mybir
import concourse.tile as tile
from concourse import bacc
from concourse.bass import ts
from concourse.bass_utils import run_bass_kernel_spmd

NCORES = 8
B, F, D = 2048, 6144, 768
F_LOC = F // NCORES  # 768
P = 128
MD = D // P          # 6 d-subtiles
DR = D // NCORES     # 96 rows per rank after ReduceScatter
BN = 512             # B block width (= matmul moving free dim, one PSUM bank)
NBLK = B // BN       # 4
CK = 8               # preferred k-tiles (of 128) per DMA chunk
WARM_MMS = 55        # dummy matmuls bridging the initial DMA wait
CK0_SPLITS_A = (4,)  # chunk-0 acts sub-chunk boundaries (k-tiles)
CK0_SPLITS_W = (4,)  # chunk-0 W sub-chunk boundaries (k-tiles)
W_ACT = 0            # W chunks [1..W_ACT] ride the ACT queue
SPLIT_CHUNKS = 2     # how many leading chunks use sub-chunk DMAs

_nc_cache = {}
last_result = None  # BassKernelResults of the most recent run (for test harness)


def _build(n_layers: int):
    K_LOC = n_layers * F_LOC          # 9216 for n=12
    KT = K_LOC // P                   # 72 k-tiles
    ck = max(c for c in (CK, 6, 4, 3, 2, 1) if KT % c == 0)
    NCH = KT // ck                    # 9 chunks for n=12

    nc = bacc.Bacc(None, num_devices=NCORES)
    a_ext = nc.dram_tensor("a_t", [K_LOC, B], mybir.dt.bfloat16, kind="ExternalInput")
    w_ext = nc.dram_tensor("w", [K_LOC, D], mybir.dt.bfloat16, kind="ExternalInput")
    b_ext = nc.dram_tensor("bias_t", [D, n_layers], mybir.dt.float32, kind="ExternalInput")
    # block-major bf16 output (host casts to fp32): lets the final copy ride
    # the idle SP HWDGE queue (no dtype conversion, which only Pool DMA does)
    y_ext = nc.dram_tensor("y", [NBLK, DR, BN], mybir.dt.bfloat16, kind="ExternalOutput")

    # bf16 wire format for the reduce: halves RS payload + partial DMAs.
    # Adds ~1e-3 quantization on partials (total rel err stays ~2.6e-3).
    partials = [nc.dram_tensor(f"partial{b}", [D, BN], mybir.dt.bfloat16) for b in range(NBLK)]
    reduceds = [nc.dram_tensor(f"reduced{b}", [DR, BN], mybir.dt.bfloat16) for b in range(NBLK)]

    a_v = a_ext[:, :].rearrange("(ko p) b -> p ko b", p=P)  # [128, KT, B]
    w_v = w_ext[:, :].rearrange("(ko p) d -> p ko d", p=P)  # [128, KT, D]
    b_v = b_ext[:, :].rearrange("(mo p) l -> p mo l", p=P)  # [128, MD, n]

    with tile.TileContext(nc) as tc:
        with (
            tc.tile_pool(name="apool", bufs=3) as apool,
            tc.tile_pool(name="wpool", bufs=NCH) as wpool,
            tc.tile_pool(name="cpool", bufs=1) as cpool,
            tc.tile_pool(name="opool", bufs=3) as opool,
            tc.tile_pool(name="pspool", bufs=8, space="PSUM") as pspool,
        ):
            # bias8[p, mo] = sum_l bias[l, mo*128+p] / NCORES  (SWDGE: keep the
            # SP queue free for the first acts chunk)
            bias_t = cpool.tile([P, MD, n_layers], mybir.dt.float32)
            nc.gpsimd.dma_start(bias_t[:], b_v)
            bias8 = cpool.tile([P, MD], mybir.dt.float32)
            nc.vector.reduce_sum(bias8[:], bias_t[:], axis=mybir.AxisListType.X)
            nc.vector.tensor_scalar_mul(bias8[:], bias8[:], 1.0 / NCORES)

            # PE warm-up: a dense dummy-matmul chain that spans the first
            # acts/W DMA wait and ends only once real data is ready, so the
            # clock ramp (half-rate for the first ~3us of PE activity; HAM on
            # real hw) is burned before the real stream starts. The chain must
            # reach the first real matmul with no PE-idle gap or the ramp
            # resets.
            scratch = cpool.tile([P, P], mybir.dt.bfloat16)
            nc.vector.memset(scratch[:], 0)
            # N=128 dummies: wide enough that the per-MM issue floor (~60
            # cycles on real hw, uncharged by the cost model) stays below the
            # streaming time, so the chain length matches on silicon too
            ps_warm = pspool.tile([P, P], mybir.dt.float32, tag="ps", name="ps_warm")
            for i in range(WARM_MMS):
                nc.tensor.matmul(
                    ps_warm[:], scratch[:], scratch[:],
                    start=(i == 0), stop=(i == WARM_MMS - 1),
                )


            def evac(blk, m, ps_m):
                """PSUM -> SBUF (+bias/8) -> partial DRAM, alternating engines."""
                ob = opool.tile([P, BN], mybir.dt.bfloat16, tag="o", name=f"ob{blk}_{m}")
                if m in (0, 2, 5):  # m5 on DVE: the ACT queue then pre-stages
                    nc.vector.tensor_scalar_add(ob[:], ps_m[:], bias8[:, m : m + 1])
                else:               # its partial-DMA dispatch during the evac
                    nc.scalar.add(ob[:], ps_m[:], bias8[:, m : m + 1])
                nc.scalar.dma_start(partials[blk][ts(m, P), :], ob[:])

            w_tiles = []
            for blk in range(NBLK):
                b0 = blk * BN
                ps = [pspool.tile([P, BN], mybir.dt.float32, tag="ps", name=f"ps{blk}_{m}") for m in range(MD)]
                for c in range(NCH):
                    a_c = apool.tile([P, ck, BN], mybir.dt.bfloat16, tag="a")
                    if blk == 0:
                        w_c = wpool.tile([P, ck, D], mybir.dt.bfloat16, tag="w")
                        w_tiles.append(w_c)
                        if c < SPLIT_CHUNKS:
                            # early chunks in two sub-chunk DMAs: matmuls wait
                            # on half a chunk, but per-DMA dispatch overhead
                            # (~1.2us) stays amortized so the stream feeds
                            # full-rate consumption. Chunk-0 W rides the ACT
                            # queue so both HWDGE queues ramp in parallel.
                            wq = nc.scalar if c == 0 else nc.sync
                            prev = 0
                            for h1 in [x for x in CK0_SPLITS_A if x < ck] + [ck]:
                                nc.sync.dma_start(a_c[:, prev:h1], a_v[:, c * ck + prev : c * ck + h1, b0 : b0 + BN])
                                prev = h1
                            prev = 0
                            for h1 in [x for x in CK0_SPLITS_W if x < ck] + [ck]:
                                wq.dma_start(w_c[:, prev:h1], w_v[:, c * ck + prev : c * ck + h1, :])
                                prev = h1
                        else:
                            # W for the first W_ACT chunks rides the ACT queue
                            # (fills DMA-device idle between SP dispatches);
                            # later chunks share the SP queue so FIFO order
                            # keeps the streams interleaved (an unbounded ACT
                            # stream floods the device and starves acts)
                            nc.sync.dma_start(a_c[:], a_v[:, c * ck : (c + 1) * ck, b0 : b0 + BN])
                            if c <= W_ACT:
                                nc.scalar.dma_start(w_c[:], w_v[:, c * ck : (c + 1) * ck, :])
                            else:
                                nc.sync.dma_start(w_c[:], w_v[:, c * ck : (c + 1) * ck, :])
                    else:
                        nc.sync.dma_start(a_c[:], a_v[:, c * ck : (c + 1) * ck, b0 : b0 + BN])
                        w_c = w_tiles[c]
                    if c < NCH - 1:
                        for k in range(ck):
                            for m in range(MD):
                                nc.tensor.matmul(
                                    ps[m][:],
                                    w_c[:, k, ts(m, P)],
                                    a_c[:, k],
                                    start=(c == 0 and k == 0),
                                    stop=False,
                                )
                    else:
                        # last chunk m-outer: each subtile's accumulation
                        # finishes early, its evacuation overlaps the rest
                        for m in range(MD):
                            for k in range(ck):
                                nc.tensor.matmul(
                                    ps[m][:],
                                    w_c[:, k, ts(m, P)],
                                    a_c[:, k],
                                    start=(c == 0 and k == 0),
                                    stop=(k == ck - 1),
                                )
                            evac(blk, m, ps[m])

                nc.gpsimd.collective_compute(
                    "ReduceScatter",
                    mybir.AluOpType.add,
                    replica_groups=[list(range(NCORES))],
                    ins=[partials[blk][:, :].opt()],
                    outs=[reduceds[blk][:, :].opt()],
                )
                if blk < NBLK - 1:
                    nc.gpsimd.dma_start(y_ext[blk], reduceds[blk][:, :])
                else:
                    # idle SP queue: dispatch pre-staged, waits only on RS sem
                    nc.sync.dma_start(y_ext[blk], reduceds[blk][:, :])
    nc.compile()
    return nc


def _get_nc(n_layers: int):
    if n_layers not in _nc_cache:
        _nc_cache[n_layers] = _build(n_layers)
    return _nc_cache[n_layers]


def kernel(acts: np.ndarray, W: np.ndarray, bias: np.ndarray, layer_idx) -> np.ndarray:
    global last_result
    n = int(layer_idx) + 1
    bf16 = ml_dtypes.bfloat16
    acts16 = np.asarray(acts, dtype=np.float32)[:n].astype(bf16)  # [n, B, F]
    W16 = np.asarray(W, dtype=np.float32)[:n].astype(bf16)        # [n, F, D]
    bias = np.asarray(bias, dtype=np.float32)[:n]                 # [n, D]

    nc = _get_nc(n)

    bias_t = np.ascontiguousarray(bias.T)  # [D, n], same on every core
    in_maps = []
    for r in range(NCORES):
        f0 = r * F_LOC
        # [n, B, F_LOC] -> [n, F_LOC, B] -> [K_loc, B]
        a_t = np.ascontiguousarray(
            acts16[:, :, f0 : f0 + F_LOC].transpose(0, 2, 1)
        ).reshape(n * F_LOC, B)
        w_r = np.ascontiguousarray(W16[:, f0 : f0 + F_LOC, :]).reshape(n * F_LOC, D)
        in_maps.append({"a_t": a_t, "w": w_r, "bias_t": bias_t})

    last_result = run_bass_kernel_spmd(nc, in_maps, core_ids=list(range(NCORES)))
    out = np.empty((D, B), dtype=np.float32)
    for r in range(NCORES):
        y_r = np.asarray(last_result.results[r]["y"]).astype(np.float32)  # [NBLK, DR, BN]
        for blk in range(NBLK):
            out[r * DR : (r + 1) * DR, blk * BN : (blk + 1) * BN] = y_r[blk]
    return np.ascontiguousarray(out.T)  # [B, D] float32


# revision 9
# speedup vs baseline: 1.0037x; 1.0025x over previous
"""Trainium2 Bass kernel for nn_Decoder_36953898615460.

recon[B, D] = einsum('lbf,lfd->bd', acts[:n], W[:n]) + sum(bias[:n], 0)

Strategy (row-parallel over F, 8 NeuronCores; TimelineSim 400.6 us vs the
563.3 us HW baseline):
  - Shard the contraction dim F across 8 cores: core r owns F columns
    [r*768, (r+1)*768)  ->  local contraction K_loc = n*768 (9216 for n=12).
  - Host prep: inputs cast to bf16 (rel err 3.8e-3 << 2e-2 tol); acts shard
    transposed to [K_loc, B]; W shard reshaped to [K_loc, D]; bias -> [D, n].
  - bf16 halves HBM traffic vs fp32: per-core DMA-in ~52 MB (~145 us) vs the
    PE floor 29 GF / 78.6 TF/s = 369 us -> cleanly PE-bound; every real
    matmul issues at the 213 ns streaming floor in the cost model.
  - W (14.2 MB bf16) stays SBUF-resident: streamed once during block 0,
    interleaved per chunk with acts on the SP HWDGE queue (FIFO pacing; an
    ungated separate W stream floods the serialized DMA engines and starves
    acts). Chunk 0 goes in two sub-chunk DMAs (first matmul waits on half a
    chunk; finer splits lose to the ~1.2 us per-DMA dispatch overhead), with
    chunk-0 W on the ACT queue so both queues ramp in parallel.
  - PE warm-up: ~55 N=128 dummy matmuls on zeroed scratch bridge the initial
    DMA wait so the clock ramp (half rate for the first ~3 us of PE activity;
    HAM on real hw) burns before the real stream starts. N=128 keeps the
    per-MM issue floor (~60 cycles on silicon, uncharged by the cost model)
    below streaming time so the chain length matches on hw.
  - B processed in 4 blocks of 512 (one PSUM bank per d-subtile). Per block,
    the full K accumulation for each of the 6 subtiles lives in a single
    PSUM bank (72 accumulating matmuls, start/stop flags) -- no SBUF
    accumulator, no per-chunk vector adds.
  - Per-block bf16-wire ReduceScatter(add) overlaps the next block's
    compute; only the last block's RS (~17.5 us, launch-constant dominated)
    is exposed at the tail. Each block's last chunk runs m-outer with the
    evacuation issued right after that subtile's stop-matmul, so the RS
    starts ~3 us after the last matmul. The final subtile evacuates on DVE
    so the ACT queue pre-stages its partial write.
  - bias: each core adds sum_l(bias)/8 during PSUM->SBUF evacuation so the
    8-way reduce sums to +bias.
  - Output y is block-major bf16 [NBLK, 96, 512] (collectives cannot write
    IO tensors, so an internal reduced buffer is copied out; bf16 lets the
    last copy ride the idle SP queue -- dtype-converting DMA is Pool-only).
    Host reassembles the 8 shards and casts to fp32.
"""

import numpy as np
import ml_dtypes

import concourse.mybir as mybir
import concourse.tile as tile
from concourse import bacc
from concourse.bass import ts
from concourse.bass_utils import run_bass_kernel_spmd

NCORES = 8
B, F, D = 2048, 6144, 768
F_LOC = F // NCORES  # 768
P = 128
MD = D // P          # 6 d-subtiles
DR = D // NCORES     # 96 rows per rank after ReduceScatter
BN = 512             # B block width (= matmul moving free dim, one PSUM bank)
NBLK = B // BN       # 4
CK = 8               # preferred k-tiles (of 128) per DMA chunk
WARM_MMS = 55        # dummy matmuls bridging the initial DMA wait
CK0_SPLITS_A = (2, 4)  # chunk-0 acts sub-chunk boundaries (k-tiles)
CK0_SPLITS_W = (2, 4)  # chunk-0 W sub-chunk boundaries (k-tiles)
W_ACT = 0            # W chunks [1..W_ACT] ride the ACT queue
SPLIT_CHUNKS = 2     # how many leading chunks use sub-chunk DMAs

_nc_cache = {}
last_result = None  # BassKernelResults of the most recent run (for test harness)


def _build(n_layers: int):
    K_LOC = n_layers * F_LOC          # 9216 for n=12
    KT = K_LOC // P                   # 72 k-tiles
    ck = max(c for c in (CK, 6, 4, 3, 2, 1) if KT % c == 0)
    NCH = KT // ck                    # 9 chunks for n=12

    nc = bacc.Bacc(None, num_devices=NCORES)
    a_ext = nc.dram_tensor("a_t", [K_LOC, B], mybir.dt.bfloat16, kind="ExternalInput")
    w_ext = nc.dram_tensor("w", [K_LOC, D], mybir.dt.bfloat16, kind="ExternalInput")
    b_ext = nc.dram_tensor("bias_t", [D, n_layers], mybir.dt.float32, kind="ExternalInput")
    # block-major bf16 output (host casts to fp32): lets the final copy ride
    # the idle SP HWDGE queue (no dtype conversion, which only Pool DMA does)
    y_ext = nc.dram_tensor("y", [NBLK, DR, BN], mybir.dt.bfloat16, kind="ExternalOutput")

    # bf16 wire format for the reduce: halves RS payload + partial DMAs.
    # Adds ~1e-3 quantization on partials (total rel err stays ~2.6e-3).
    partials = [nc.dram_tensor(f"partial{b}", [D, BN], mybir.dt.bfloat16) for b in range(NBLK)]
    reduceds = [nc.dram_tensor(f"reduced{b}", [DR, BN], mybir.dt.bfloat16) for b in range(NBLK)]

    a_v = a_ext[:, :].rearrange("(ko p) b -> p ko b", p=P)  # [128, KT, B]
    w_v = w_ext[:, :].rearrange("(ko p) d -> p ko d", p=P)  # [128, KT, D]
    b_v = b_ext[:, :].rearrange("(mo p) l -> p mo l", p=P)  # [128, MD, n]

    with tile.TileContext(nc) as tc:
        with (
            tc.tile_pool(name="apool", bufs=3) as apool,
            tc.tile_pool(name="wpool", bufs=NCH) as wpool,
            tc.tile_pool(name="cpool", bufs=1) as cpool,
            tc.tile_pool(name="opool", bufs=3) as opool,
            tc.tile_pool(name="pspool", bufs=8, space="PSUM") as pspool,
        ):
            # bias8[p, mo] = sum_l bias[l, mo*128+p] / NCORES  (SWDGE: keep the
            # SP queue free for the first acts chunk)
            bias_t = cpool.tile([P, MD, n_layers], mybir.dt.float32)
            nc.gpsimd.dma_start(bias_t[:], b_v)
            bias8 = cpool.tile([P, MD], mybir.dt.float32)
            nc.vector.reduce_sum(bias8[:], bias_t[:], axis=mybir.AxisListType.X)
            nc.vector.tensor_scalar_mul(bias8[:], bias8[:], 1.0 / NCORES)

            # PE warm-up: a dense dummy-matmul chain that spans the first
            # acts/W DMA wait and ends only once real data is ready, so the
            # clock ramp (half-rate for the first ~3us of PE activity; HAM on
            # real hw) is burned before the real stream starts. The chain must
            # reach the first real matmul with no PE-idle gap or the ramp
            # resets.
            scratch = cpool.tile([P, P], mybir.dt.bfloat16)
            nc.vector.memset(scratch[:], 0)
            # N=128 dummies: wide enough that the per-MM issue floor (~60
            # cycles on real hw, uncharged by the cost model) stays below the
            # streaming time, so the chain length matches on silicon too
            ps_warm = pspool.tile([P, P], mybir.dt.float32, tag="ps", name="ps_warm")
            for i in range(WARM_MMS):
                nc.tensor.matmul(
                    ps_warm[:], scratch[:], scratch[:],
                    start=(i == 0), stop=(i == WARM_MMS - 1),
                )


            def evac(blk, m, ps_m):
                """PSUM -> SBUF (+bias/8) -> partial DRAM, alternating engines."""
                ob = opool.tile([P, BN], mybir.dt.bfloat16, tag="o", name=f"ob{blk}_{m}")
                if m in (0, 2, 5):  # m5 on DVE: the ACT queue then pre-stages
                    nc.vector.tensor_scalar_add(ob[:], ps_m[:], bias8[:, m : m + 1])
                else:               # its partial-DMA dispatch during the evac
                    nc.scalar.add(ob[:], ps_m[:], bias8[:, m : m + 1])
                nc.scalar.dma_start(partials[blk][ts(m, P), :], ob[:])

            w_tiles = []
            for blk in range(NBLK):
                b0 = blk * BN
                ps = [pspool.tile([P, BN], mybir.dt.float32, tag="ps", name=f"ps{blk}_{m}") for m in range(MD)]
                for c in range(NCH):
                    a_c = apool.tile([P, ck, BN], mybir.dt.bfloat16, tag="a")
                    if blk == 0:
                        w_c = wpool.tile([P, ck, D], mybir.dt.bfloat16, tag="w")
                        w_tiles.append(w_c)
                        if c < SPLIT_CHUNKS:
                            # early chunks in two sub-chunk DMAs: matmuls wait
                            # on half a chunk, but per-DMA dispatch overhead
                            # (~1.2us) stays amortized so the stream feeds
                            # full-rate consumption. Chunk-0 W rides the ACT
                            # queue so both HWDGE queues ramp in parallel.
                            wq = nc.scalar if c == 0 else nc.sync
                            prev = 0
                            for h1 in [x for x in CK0_SPLITS_A if x < ck] + [ck]:
                                nc.sync.dma_start(a_c[:, prev:h1], a_v[:, c * ck + prev : c * ck + h1, b0 : b0 + BN])
                                prev = h1
                            prev = 0
                            for h1 in [x for x in CK0_SPLITS_W if x < ck] + [ck]:
                                wq.dma_start(w_c[:, prev:h1], w_v[:, c * ck + prev : c * ck + h1, :])
                                prev = h1
                        else:
                            # W for the first W_ACT chunks rides the ACT queue
                            # (fills DMA-device idle between SP dispatches);
                            # later chunks share the SP queue so FIFO order
                            # keeps the streams interleaved (an unbounded ACT
                            # stream floods the device and starves acts)
                            nc.sync.dma_start(a_c[:], a_v[:, c * ck : (c + 1) * ck, b0 : b0 + BN])
                            if c <= W_ACT:
                                nc.scalar.dma_start(w_c[:], w_v[:, c * ck : (c + 1) * ck, :])
                            else:
                                nc.sync.dma_start(w_c[:], w_v[:, c * ck : (c + 1) * ck, :])
                    else:
                        nc.sync.dma_start(a_c[:], a_v[:, c * ck : (c + 1) * ck, b0 : b0 + BN])
                        w_c = w_tiles[c]
                    if c < NCH - 1:
                        for k in range(ck):
                            for m in range(MD):
                                nc.tensor.matmul(
                                    ps[m][:],
                                    w_c[:, k, ts(m, P)],
                                    a_c[:, k],
                                    start=(c == 0 and k == 0),
                                    stop=False,
                                )
                    else:
                        # last chunk m-outer: each subtile's accumulation
                        # finishes early, its evacuation overlaps the rest
                        for m in range(MD):
                            for k in range(ck):
                                nc.tensor.matmul(
                                    ps[m][:],
                                    w_c[:, k, ts(m, P)],
                                    a_c[:, k],
                                    start=(c == 0 and k == 0),
                                    stop=(k == ck - 1),
                                )
                            evac(blk, m, ps[m])

                nc.gpsimd.collective_compute(
                    "ReduceScatter",
                    mybir.AluOpType.add,
                    replica_groups=[list(range(NCORES))],
                    ins=[partials[blk][:, :].opt()],
                    outs=[reduceds[blk][:, :].opt()],
                )
                if blk < NBLK - 1:
                    nc.gpsimd.dma_start(y_ext[blk], reduceds[blk][:, :])
                else:
                    # idle SP queue: dispatch pre-staged, waits only on RS sem
                    nc.sync.dma_start(y_ext[blk], reduceds[blk][:, :])
    nc.compile()
    return nc


def _get_nc(n_layers: int):
    if n_layers not in _nc_cache:
        _nc_cache[n_layers] = _build(n_layers)
    return _nc_cache[n_layers]


def kernel(acts: np.ndarray, W: np.ndarray, bias: np.ndarray, layer_idx) -> np.ndarray:
    global last_result
    n = int(layer_idx) + 1
    bf16 = ml_dtypes.bfloat16
    acts16 = np.asarray(acts, dtype=np.float32)[:n].astype(bf16)  # [n, B, F]
    W16 = np.asarray(W, dtype=np.float32)[:n].astype(bf16)        # [n, F, D]
    bias = np.asarray(bias, dtype=np.float32)[:n]                 # [n, D]

    nc = _get_nc(n)

    bias_t = np.ascontiguousarray(bias.T)  # [D, n], same on every core
    in_maps = []
    for r in range(NCORES):
        f0 = r * F_LOC
        # [n, B, F_LOC] -> [n, F_LOC, B] -> [K_loc, B]
        a_t = np.ascontiguousarray(
            acts16[:, :, f0 : f0 + F_LOC].transpose(0, 2, 1)
        ).reshape(n * F_LOC, B)
        w_r = np.ascontiguousarray(W16[:, f0 : f0 + F_LOC, :]).reshape(n * F_LOC, D)
        in_maps.append({"a_t": a_t, "w": w_r, "bias_t": bias_t})

    last_result = run_bass_kernel_spmd(nc, in_maps, core_ids=list(range(NCORES)))
    out = np.empty((D, B), dtype=np.float32)
    for r in range(NCORES):
        y_r = np.asarray(last_result.results[r]["y"]).astype(np.float32)  # [NBLK, DR, BN]
        for blk in range(NBLK):
            out[r * DR : (r + 1) * DR, blk * BN : (blk + 1) * BN] = y_r[blk]
    return np.ascontiguousarray(out.T)  # [B, D] float32
